# revision 2
# baseline (speedup 1.0000x reference)
"""Bahdanau (additive) attention kernel for Trainium2, 8 NeuronCores.

Full-input contract: kernel(**inputs) takes the unsharded numpy inputs and
returns the full [TQ, B, D] output. Internally shards (batch, query-half)
across 8 cores (B=4 x 2 halves of Tq), runs a Bass/Tile kernel per core via
run_bass_kernel_spmd, and reassembles.

Algorithmic core: the additive score
    scores[q,v] = sum_u s_u tanh(wq[q,u] + wk[v,u])
is evaluated via a fitted LOW-RANK SEPARABLE expansion
    tanh(a+b) ~= sum_k c_k tanh(ga_k a + da_k) * tanh(gb_k b + db_k)
Each feature contributes one accumulating PE matmul into the score PSUM.
This replaces the O(TQL*TVE*U) tanh evaluation with O(R*(TQL+TVE))
activations + O(R) matmuls; the ACT engine is the bottleneck, so the
schedule keeps it saturated: all q-side atoms run first (while the vt DMA
and wk matmul complete off the critical path), then the v-side atoms, each
immediately feeding its score matmul.

Sparsity: masked value positions are gathered on the host (mask is input
data), padded to a common TVE. Padded columns need no score penalty: their
value rows AND normalizer-column entries in vnp are zero, so whatever
exp(score) they produce contributes nothing to ctx or the normalizer.

Softmax tail: chunk-pipelined across engines -- per 128-column chunk the
score PSUM is exp'ed (ACT), PE-transposed, copied to SBUF (DVE), and
accumulated into ctx via PE matmul against [v | 1]; the last ctx column
yields the softmax normalizer consistently; DVE reciprocal + per-partition
scale on output.
"""

import sys

if "/opt/trn_rl_repo" not in sys.path:
    sys.path.insert(0, "/opt/trn_rl_repo")

import numpy as np

TQ, TV, B, D, U = 256, 1024, 4, 128, 128
NCORES = 8
TQL = 128
NEG_INF = -1e9

# Fitted separable expansion (filled from fit_structured.py):
# v-side shared atoms (gb, db)
VATOMS = [
    (0.47444510, 1.39766216),
    (0.48982596, 0.07451601),
    (1.37957418, -0.07826442),
    (1.31198537, -1.28328967),
    (1.06376755, 1.44153440),
    (0.87626874, -0.34579548),
    (1.11406505, -0.14585873),
    (1.36467373, -1.28521180),
    (-0.67345953, -2.40418053),
    (0.95521957, -2.94330263),
    (-0.61966121, 1.94561911),
    (0.16753665, 0.11532875),
    (2.26979327, 1.33731830),
    (2.12410307, 1.39618659),
    (2.58440638, 2.12091875),
    (2.37144637, 1.95568621),
    (0.38434589, -0.31339884),
    (3.02359366, -2.61771035),
    (2.54027200, -2.07246137),
]
# features: (c, ga, da, vspec); vspec = int atom idx | (i, j) atom product
FEATS = [
    (1.70959640, 0.25151360, 0.03138908, 0),
    (2.92134929, -0.43775776, 1.09167695, 1),
    (-0.77592200, -1.55171931, -1.17404127, 2),
    (0.96599334, -1.30268359, -0.93301958, 3),
    (0.53638780, 1.80897105, -1.21104717, 4),
    (1.60405707, -0.90852302, 0.42689738, 5),
    (-1.07496023, -0.62484956, 1.85881972, 6),
    (-1.96126282, -0.37054500, -0.16805789, 7),
    (-0.35367522, -1.32026529, -0.79605997, 8),
    (-0.87735492, 0.42294371, -0.19769210, 9),
    (-0.67645180, 1.32222283, -0.76137090, 10),
    (0.69646239, 1.75380993, -2.25412583, 11),
    (0.57765937, -0.71790487, -0.03128870, 12),
    (0.69881064, 1.59039760, -0.02415021, 13),
    (0.86268389, -0.23864533, -0.29861382, 14),
    (0.36266690, 1.05495822, 1.55348122, 15),
    (0.31275606, -2.06651807, -2.63506889, 16),
    (-0.25702652, 0.14892404, 0.46942550, 17),
    (-0.23037328, -1.92254341, -2.47092772, 18),
]

_CACHE = {}


def _bank_pieces(tve):
    """Split [0, tve) into PSUM-bank-aligned matmul slices (<=512 each)."""
    pieces = []
    a = 0
    while a < tve:
        n = min(512, tve - a)
        pieces.append((a, n))
        a += n
    return pieces


def _build_nc(tve):
    import concourse.bacc as bacc
    import concourse.mybir as mybir
    import concourse.tile as tile
    from contextlib import ExitStack

    f32 = mybir.dt.float32
    f32r = mybir.dt.float32r
    bf16 = mybir.dt.bfloat16
    AFT = mybir.ActivationFunctionType

    nc = bacc.Bacc("TRN2", target_bir_lowering=False, debug=False,
                   num_devices=NCORES)

    R = len(FEATS)
    NVC = -(-tve // 128)
    pieces = _bank_pieces(tve)

    wpack = nc.dram_tensor("wpack", [D, 3 * 128], f32r,
                           kind="ExternalInput").ap()
    vt = nc.dram_tensor("vt", [D, tve], f32r, kind="ExternalInput").ap()
    vnp = nc.dram_tensor("vnp", [128, NVC * (D + 1)], f32,
                         kind="ExternalInput").ap()
    NA = len(VATOMS)
    # columns: R x (c_k*s) | R x da_k | NA x db_i
    csp = nc.dram_tensor("csp", [U, 2 * R + NA], f32,
                         kind="ExternalInput").ap()
    ident = nc.dram_tensor("ident", [128, 128], f32,
                           kind="ExternalInput").ap()
    out = nc.dram_tensor("out", [TQL, D], f32, kind="ExternalOutput").ap()

    with tile.TileContext(nc) as tc:
        with ExitStack() as ctx:
            consts = ctx.enter_context(tc.tile_pool(name="consts", bufs=1))
            uap = ctx.enter_context(tc.tile_pool(name="ua", bufs=3))
            ps1 = ctx.enter_context(tc.tile_pool(name="ps1", bufs=1,
                                                 space="PSUM"))
            pst = ctx.enter_context(tc.tile_pool(name="pst", bufs=1,
                                                 space="PSUM"))

            wpack_sb = consts.tile([D, 3 * 128], f32r, tag="wpack")
            w1_sb = wpack_sb[:, 0:128]
            qt_sb = wpack_sb[:, 128:256]
            w2_sb = wpack_sb[:, 256:384]
            vt_sb = consts.tile([D, tve], f32r, tag="vt")
            vnp_sb = consts.tile([128, NVC * (D + 1)], f32, tag="vnp")
            csp_sb = consts.tile([U, 2 * R + NA], f32, tag="csp")
            id_sb = consts.tile([128, 128], f32, tag="id")
            vb_sb = consts.tile([U, R * tve], bf16, tag="vb")
            lh_sb = consts.tile([U, R * TQL], bf16, tag="lh")

            # preload the exp/tanh ACT table during the input DMAs
            warm_in = consts.tile([128, 1], f32, tag="warm_in")
            warm_out = consts.tile([128, 1], f32, tag="warm_out")
            nc.vector.memset(warm_in[:], 0.0)
            nc.scalar.activation(warm_out[:], warm_in[:], AFT.Tanh)

            # DMA routing: wq path (wpack) first on the sync HWDGE queue,
            # vt behind it (first consumed ~3us later); csp on the scalar
            # queue (overlaps the ACT table load); tail-only tensors (vnp,
            # ident) on the gpsimd SWDGE queue, off every critical path.
            nc.sync.dma_start(wpack_sb[:], wpack[:])
            nc.sync.dma_start(vt_sb[:], vt[:])
            nc.scalar.dma_start(csp_sb[:], csp[:])
            nc.gpsimd.dma_start(vnp_sb[:], vnp[:])
            nc.gpsimd.dma_start(id_sb[:], ident[:])

            # wqT[u,q] and wkT[u,v] stay in PSUM (ACT reads PSUM cheaply)
            wq_ps = ps1.tile([U, TQL], f32, tag="wq")
            nc.tensor.matmul(wq_ps[:], lhsT=w1_sb[:], rhs=qt_sb[:])
            wk_ps = ps1.tile([U, tve], f32, tag="wk")
            for a, n in pieces:
                nc.tensor.matmul(wk_ps[:, a:a + n], lhsT=w2_sb[:],
                                 rhs=vt_sb[:, a:a + n])

            # all q-side atoms first: keeps ACT busy while vt/wk complete;
            # bf16 out (ACT cost is dtype-independent, DVE gets fast mode)
            for k, (c_k, ga, da, vs) in enumerate(FEATS):
                ua = uap.tile([U, TQL], bf16, tag="ua")
                nc.scalar.activation(ua[:], wq_ps[:], AFT.Tanh,
                                     bias=csp_sb[:, R + k:R + k + 1],
                                     scale=float(ga))
                nc.vector.tensor_scalar_mul(lh_sb[:, k * TQL:(k + 1) * TQL],
                                            ua[:], csp_sb[:, k:k + 1])

            # v-side atoms, each immediately feeding its score matmuls
            scores_ps = ps1.tile([TQL, tve], f32, tag="scores")
            for k, (c_k, ga, da, vs) in enumerate(FEATS):
                vb_k = vb_sb[:, k * tve:(k + 1) * tve]
                if isinstance(vs, tuple):
                    i, j = vs
                    nc.vector.tensor_mul(
                        vb_k, vb_sb[:, i * tve:(i + 1) * tve],
                        vb_sb[:, j * tve:(j + 1) * tve])
                else:
                    gb, db = VATOMS[vs]
                    nc.scalar.activation(
                        vb_k, wk_ps[:], AFT.Tanh,
                        bias=csp_sb[:, 2 * R + vs:2 * R + vs + 1],
                        scale=float(gb))
                lw = lh_sb[:, k * TQL:(k + 1) * TQL]
                for a, n in pieces:
                    nc.tensor.matmul(scores_ps[:, a:a + n], lhsT=lw,
                                     rhs=vb_sb[:, k * tve + a:k * tve + a + n],
                                     start=(k == 0), stop=(k == R - 1),
                                     skip_group_check=True)

            # softmax tail, chunk-pipelined: exp (ACT) -> transpose (PE) ->
            # copy to SBUF (DVE) -> ctx matmul vs [v | 1] (PE, accumulating);
            # the ones column gives the softmax normalizer consistently
            exp_sb = consts.tile([TQL, NVC * 128], f32, tag="exp")
            tp_all = pst.tile([128, NVC * 128], f32, tag="tpa")
            et_all = consts.tile([128, NVC * 128], f32, tag="eta")
            ctx_ps = ps1.tile([TQL, D + 1], f32, tag="ctx")
            for kc in range(NVC):
                n = min(128, tve - kc * 128)
                c0 = kc * 128
                nc.scalar.activation(exp_sb[:, c0:c0 + n],
                                     scores_ps[:, c0:c0 + n], AFT.Exp)
                nc.tensor.transpose(tp_all[:n, c0:c0 + 128],
                                    exp_sb[:, c0:c0 + n], id_sb[:])
                nc.vector.tensor_copy(et_all[:n, c0:c0 + 128],
                                      tp_all[:n, c0:c0 + 128])
            for kc in range(NVC):
                n = min(128, tve - kc * 128)
                nc.tensor.matmul(
                    ctx_ps[:], lhsT=et_all[:n, kc * 128:kc * 128 + 128],
                    rhs=vnp_sb[:n, kc * (D + 1):(kc + 1) * (D + 1)],
                    start=(kc == 0), stop=(kc == NVC - 1))

            rins = consts.tile([TQL, 1], f32, tag="rins")
            nc.vector.reciprocal(rins[:], ctx_ps[:, D:D + 1])
            out_sb = consts.tile([TQL, D], f32, tag="out")
            nc.vector.tensor_scalar_mul(out_sb[:], ctx_ps[:, 0:D], rins[:])
            nc.sync.dma_start(out[:], out_sb[:])

    nc.compile()
    return nc


def get_nc(tve=TV):
    key = ("nc", tve)
    if key not in _CACHE:
        _CACHE[key] = _build_nc(tve)
    return _CACHE[key]


def prep_in_maps(query, value, mask, W1, W2, scale):
    """Gather valid value positions per batch; returns (in_maps, tve)."""
    query = np.asarray(query, dtype=np.float32)
    value = np.asarray(value, dtype=np.float32)
    mask = np.asarray(mask)
    W1 = np.ascontiguousarray(np.asarray(W1, dtype=np.float32))
    W2 = np.ascontiguousarray(np.asarray(W2, dtype=np.float32))
    scale = np.asarray(scale, dtype=np.float32)

    R = len(FEATS)
    NA = len(VATOMS)
    idxs = [np.nonzero(mask[:, b])[0] for b in range(B)]
    nv_max = max(1, max(len(ix) for ix in idxs))
    tve = min(TV, -(-nv_max // 4) * 4)
    NVC = -(-tve // 128)

    ident = np.eye(128, dtype=np.float32)
    csp = np.zeros((U, 2 * R + NA), np.float32)
    for k, f in enumerate(FEATS):
        csp[:, k] = scale * f[0]      # (c_k * s) lhsT scale
        csp[:, R + k] = f[2]          # da_k bias
    for i, (gb, db) in enumerate(VATOMS):
        csp[:, 2 * R + i] = db        # db_i bias
    csp = np.ascontiguousarray(csp)

    in_maps = []
    for c in range(NCORES):
        b, q0 = c // 2, (c % 2) * TQL
        ix = idxs[b]
        nv = len(ix)
        # padded rows are zero in BOTH the value block and the normalizer
        # column, so they contribute nothing regardless of their score
        vg = np.zeros((NVC * 128, D + 1), np.float32)
        vg[:nv, :D] = value[ix, b, :]
        vg[:nv, D] = 1.0
        wpack = np.concatenate(
            [W1, np.ascontiguousarray(query[q0:q0 + TQL, b, :].T), W2],
            axis=1)
        in_maps.append({
            "wpack": np.ascontiguousarray(wpack),
            "vt": np.ascontiguousarray(vg[:tve, :D].T),
            "vnp": np.ascontiguousarray(
                vg.reshape(NVC, 128, D + 1).transpose(1, 0, 2)
                .reshape(128, NVC * (D + 1))),
            "csp": csp,
            "ident": ident,
        })
    return in_maps, tve


def run(query, value, mask, W1, W2, scale, trace=False):
    from concourse.bass_utils import run_bass_kernel_spmd

    in_maps, tve = prep_in_maps(query, value, mask, W1, W2, scale)
    nc = get_nc(tve)
    res = run_bass_kernel_spmd(nc, in_maps, list(range(NCORES)), trace=trace)
    out = np.empty((TQ, B, D), np.float32)
    for c in range(NCORES):
        b, q0 = c // 2, (c % 2) * TQL
        out[q0:q0 + TQL, b, :] = res.results[c]["out"]
    return out, res


def kernel(query, value, mask, W1, W2, scale):
    out, _ = run(query, value, mask, W1, W2, scale, trace=False)
    return out


# revision 9
# speedup vs baseline: 1.2248x; 1.2248x over previous
"""Bahdanau (additive) attention kernel for Trainium2, 8 NeuronCores.

Full-input contract: kernel(**inputs) takes the unsharded numpy inputs and
returns the full [TQ, B, D] output. Internally shards (batch, query-half)
across 8 cores (B=4 x 2 halves of Tq), runs a Bass/Tile kernel per core via
run_bass_kernel_spmd, and reassembles.

Algorithmic core: the additive score
    scores[q,v] = sum_u s_u tanh(wq[q,u] + wk[v,u])
is evaluated via a fitted LOW-RANK SEPARABLE expansion
    tanh(a+b) ~= sum_k c_k tanh(ga_k a + da_k) * tanh(gb_k b + db_k)
Each feature contributes one accumulating PE matmul into the score PSUM.
This replaces the O(TQL*TVE*U) tanh evaluation with O(R*(TQL+TVE))
activations + O(R) matmuls; the ACT engine is the bottleneck, so the
schedule keeps it saturated: all q-side atoms run first (while the vt DMA
and wk matmul complete off the critical path), then the v-side atoms, each
immediately feeding its score matmul.

Sparsity: masked value positions are gathered on the host (mask is input
data), padded to a common TVE. Padded columns need no score penalty: their
value rows AND normalizer-column entries in vnp are zero, so whatever
exp(score) they produce contributes nothing to ctx or the normalizer.

Softmax tail: chunk-pipelined across engines -- per 128-column chunk the
score PSUM is exp'ed (ACT), PE-transposed, copied to SBUF (DVE), and
accumulated into ctx via PE matmul against [v | 1]; the last ctx column
yields the softmax normalizer consistently; DVE reciprocal + per-partition
scale on output.
"""

import sys

if "/opt/trn_rl_repo" not in sys.path:
    sys.path.insert(0, "/opt/trn_rl_repo")

import numpy as np

TQ, TV, B, D, U = 256, 1024, 4, 128, 128
NCORES = 8
TQL = 128
NEG_INF = -1e9

# Fitted separable expansion (filled from fit_structured.py):
# v-side shared atoms (gb, db)
VATOMS = [
    (0.47444510, 1.39766216),
    (0.48982596, 0.07451601),
    (1.37957418, -0.07826442),
    (1.31198537, -1.28328967),
    (1.06376755, 1.44153440),
    (0.87626874, -0.34579548),
    (1.11406505, -0.14585873),
    (1.36467373, -1.28521180),
    (-0.67345953, -2.40418053),
    (0.95521957, -2.94330263),
    (-0.61966121, 1.94561911),
    (0.16753665, 0.11532875),
    (2.26979327, 1.33731830),
    (2.12410307, 1.39618659),
    (2.58440638, 2.12091875),
    (2.37144637, 1.95568621),
    (0.38434589, -0.31339884),
    (3.02359366, -2.61771035),
    (2.54027200, -2.07246137),
]
# features: (c, ga, da, vspec); vspec = int atom idx | (i, j) atom product
FEATS = [
    (1.70959640, 0.25151360, 0.03138908, 0),
    (2.92134929, -0.43775776, 1.09167695, 1),
    (-0.77592200, -1.55171931, -1.17404127, 2),
    (0.96599334, -1.30268359, -0.93301958, 3),
    (0.53638780, 1.80897105, -1.21104717, 4),
    (1.60405707, -0.90852302, 0.42689738, 5),
    (-1.07496023, -0.62484956, 1.85881972, 6),
    (-1.96126282, -0.37054500, -0.16805789, 7),
    (-0.35367522, -1.32026529, -0.79605997, 8),
    (-0.87735492, 0.42294371, -0.19769210, 9),
    (-0.67645180, 1.32222283, -0.76137090, 10),
    (0.69646239, 1.75380993, -2.25412583, 11),
    (0.57765937, -0.71790487, -0.03128870, 12),
    (0.69881064, 1.59039760, -0.02415021, 13),
    (0.86268389, -0.23864533, -0.29861382, 14),
    (0.36266690, 1.05495822, 1.55348122, 15),
    (0.31275606, -2.06651807, -2.63506889, 16),
    (-0.25702652, 0.14892404, 0.46942550, 17),
    (-0.23037328, -1.92254341, -2.47092772, 18),
]

_CACHE = {}


def _bank_pieces(tve):
    """Split [0, tve) into PSUM-bank-aligned matmul slices (<=512 each)."""
    pieces = []
    a = 0
    while a < tve:
        n = min(512, tve - a)
        pieces.append((a, n))
        a += n
    return pieces


def _build_nc(tve):
    import concourse.bacc as bacc
    import concourse.mybir as mybir
    import concourse.tile as tile
    from contextlib import ExitStack

    f32 = mybir.dt.float32
    f32r = mybir.dt.float32r
    bf16 = mybir.dt.bfloat16
    AFT = mybir.ActivationFunctionType

    nc = bacc.Bacc("TRN2", target_bir_lowering=False, debug=False,
                   num_devices=NCORES)

    R = len(FEATS)
    NVC = -(-tve // 128)
    pieces = _bank_pieces(tve)

    wpack = nc.dram_tensor("wpack", [D, 3 * 128], f32r,
                           kind="ExternalInput").ap()
    vt = nc.dram_tensor("vt", [D, tve], f32r, kind="ExternalInput").ap()
    vnp = nc.dram_tensor("vnp", [128, NVC * (D + 1)], bf16,
                         kind="ExternalInput").ap()
    NA = len(VATOMS)
    # columns: R x (c_k*s) | R x da_k | NA x db_i
    csp = nc.dram_tensor("csp", [U, 2 * R + NA], f32,
                         kind="ExternalInput").ap()
    ident = nc.dram_tensor("ident", [128, 128], bf16,
                           kind="ExternalInput").ap()
    out = nc.dram_tensor("out", [TQL, D], f32, kind="ExternalOutput").ap()

    with tile.TileContext(nc) as tc:
        with ExitStack() as ctx:
            consts = ctx.enter_context(tc.tile_pool(name="consts", bufs=1))
            uap = ctx.enter_context(tc.tile_pool(name="ua", bufs=3))
            ps1 = ctx.enter_context(tc.tile_pool(name="ps1", bufs=1,
                                                 space="PSUM"))
            pst = ctx.enter_context(tc.tile_pool(name="pst", bufs=1,
                                                 space="PSUM"))

            wpack_sb = consts.tile([D, 3 * 128], f32r, tag="wpack")
            w1_sb = wpack_sb[:, 0:128]
            qt_sb = wpack_sb[:, 128:256]
            w2_sb = wpack_sb[:, 256:384]
            vt_sb = consts.tile([D, tve], f32r, tag="vt")
            vnp_sb = consts.tile([128, NVC * (D + 1)], bf16, tag="vnp")
            csp_sb = consts.tile([U, 2 * R + NA], f32, tag="csp")
            id_sb = consts.tile([128, 128], bf16, tag="id")
            vb_sb = consts.tile([U, R * tve], bf16, tag="vb")
            lh_sb = consts.tile([U, R * TQL], bf16, tag="lh")

            # preload the exp/tanh ACT table during the input DMAs
            warm_in = consts.tile([128, 1], f32, tag="warm_in")
            warm_out = consts.tile([128, 1], f32, tag="warm_out")
            nc.vector.memset(warm_in[:], 0.0)
            nc.scalar.activation(warm_out[:], warm_in[:], AFT.Tanh)

            # DMA routing: wq path (wpack) first on the sync HWDGE queue,
            # vt behind it (first consumed ~3us later); csp on the scalar
            # queue (overlaps the ACT table load); tail-only tensors (vnp,
            # ident) on the gpsimd SWDGE queue, off every critical path.
            nc.sync.dma_start(wpack_sb[:], wpack[:])
            nc.sync.dma_start(vt_sb[:], vt[:])
            nc.scalar.dma_start(csp_sb[:], csp[:])
            nc.gpsimd.dma_start(vnp_sb[:], vnp[:])
            nc.gpsimd.dma_start(id_sb[:], ident[:])

            # wqT[u,q] and wkT[u,v] stay in PSUM (ACT reads PSUM cheaply)
            wq_ps = ps1.tile([U, TQL], f32, tag="wq")
            nc.tensor.matmul(wq_ps[:], lhsT=w1_sb[:], rhs=qt_sb[:])
            wk_ps = ps1.tile([U, tve], f32, tag="wk")
            for a, n in pieces:
                nc.tensor.matmul(wk_ps[:, a:a + n], lhsT=w2_sb[:],
                                 rhs=vt_sb[:, a:a + n])

            # all q-side atoms first: keeps ACT busy while vt/wk complete;
            # ua stays f32 so lh sees only one bf16 rounding (accuracy)
            for k, (c_k, ga, da, vs) in enumerate(FEATS):
                ua = uap.tile([U, TQL], f32, tag="ua")
                nc.scalar.activation(ua[:], wq_ps[:], AFT.Tanh,
                                     bias=csp_sb[:, R + k:R + k + 1],
                                     scale=float(ga))
                nc.vector.tensor_scalar_mul(lh_sb[:, k * TQL:(k + 1) * TQL],
                                            ua[:], csp_sb[:, k:k + 1])

            # v-side atoms, each immediately feeding its score matmuls
            scores_ps = ps1.tile([TQL, tve], f32, tag="scores")
            for k, (c_k, ga, da, vs) in enumerate(FEATS):
                vb_k = vb_sb[:, k * tve:(k + 1) * tve]
                if isinstance(vs, tuple):
                    i, j = vs
                    nc.vector.tensor_mul(
                        vb_k, vb_sb[:, i * tve:(i + 1) * tve],
                        vb_sb[:, j * tve:(j + 1) * tve])
                else:
                    gb, db = VATOMS[vs]
                    nc.scalar.activation(
                        vb_k, wk_ps[:], AFT.Tanh,
                        bias=csp_sb[:, 2 * R + vs:2 * R + vs + 1],
                        scale=float(gb))
                lw = lh_sb[:, k * TQL:(k + 1) * TQL]
                for a, n in pieces:
                    nc.tensor.matmul(scores_ps[:, a:a + n], lhsT=lw,
                                     rhs=vb_sb[:, k * tve + a:k * tve + a + n],
                                     start=(k == 0), stop=(k == R - 1),
                                     skip_group_check=True)

            # softmax tail: piece-wise exp to bf16 (wide ACT instrs amortize
            # the ~200ns ACT overhead), then per-128-chunk bf16 transpose
            # (PE, 1 cyc/row) -> copy to SBUF (DVE 2x mode) -> single-pass
            # bf16 ctx matmul vs [v | 1] (f32 rhs would need 2 PE passes);
            # the ones column gives the softmax normalizer consistently
            exp_sb = consts.tile([TQL, NVC * 128], bf16, tag="exp")
            tp_all = pst.tile([128, NVC * 128], bf16, tag="tpa")
            et_all = consts.tile([128, NVC * 128], bf16, tag="eta")
            ctx_ps = ps1.tile([TQL, D + 1], f32, tag="ctx")
            for a, n in pieces:
                nc.scalar.activation(exp_sb[:, a:a + n],
                                     scores_ps[:, a:a + n], AFT.Exp)
            for kc in range(NVC):
                n = min(128, tve - kc * 128)
                c0 = kc * 128
                nc.tensor.transpose(tp_all[:n, c0:c0 + 128],
                                    exp_sb[:, c0:c0 + n], id_sb[:])
                nc.vector.tensor_copy(et_all[:n, c0:c0 + 128],
                                      tp_all[:n, c0:c0 + 128])
            for kc in range(NVC):
                n = min(128, tve - kc * 128)
                nc.tensor.matmul(
                    ctx_ps[:], lhsT=et_all[:n, kc * 128:kc * 128 + 128],
                    rhs=vnp_sb[:n, kc * (D + 1):(kc + 1) * (D + 1)],
                    start=(kc == 0), stop=(kc == NVC - 1))

            rins = consts.tile([TQL, 1], f32, tag="rins")
            nc.vector.reciprocal(rins[:], ctx_ps[:, D:D + 1])
            out_sb = consts.tile([TQL, D], f32, tag="out")
            nc.vector.tensor_scalar_mul(out_sb[:], ctx_ps[:, 0:D], rins[:])
            nc.sync.dma_start(out[:], out_sb[:])

    nc.compile()
    return nc


def get_nc(tve=TV):
    key = ("nc", tve)
    if key not in _CACHE:
        _CACHE[key] = _build_nc(tve)
    return _CACHE[key]


def prep_in_maps(query, value, mask, W1, W2, scale):
    """Gather valid value positions per batch; returns (in_maps, tve)."""
    query = np.asarray(query, dtype=np.float32)
    value = np.asarray(value, dtype=np.float32)
    mask = np.asarray(mask)
    W1 = np.ascontiguousarray(np.asarray(W1, dtype=np.float32))
    W2 = np.ascontiguousarray(np.asarray(W2, dtype=np.float32))
    scale = np.asarray(scale, dtype=np.float32)

    R = len(FEATS)
    NA = len(VATOMS)
    idxs = [np.nonzero(mask[:, b])[0] for b in range(B)]
    nv_max = max(1, max(len(ix) for ix in idxs))
    tve = min(TV, -(-nv_max // 4) * 4)
    NVC = -(-tve // 128)

    import ml_dtypes
    bf16_np = np.dtype(ml_dtypes.bfloat16)
    ident = np.eye(128, dtype=bf16_np)
    csp = np.zeros((U, 2 * R + NA), np.float32)
    for k, f in enumerate(FEATS):
        csp[:, k] = scale * f[0]      # (c_k * s) lhsT scale
        csp[:, R + k] = f[2]          # da_k bias
    for i, (gb, db) in enumerate(VATOMS):
        csp[:, 2 * R + i] = db        # db_i bias
    csp = np.ascontiguousarray(csp)

    in_maps = []
    for c in range(NCORES):
        b, q0 = c // 2, (c % 2) * TQL
        ix = idxs[b]
        nv = len(ix)
        # padded rows are zero in BOTH the value block and the normalizer
        # column, so they contribute nothing regardless of their score
        vg = np.zeros((NVC * 128, D + 1), np.float32)
        vg[:nv, :D] = value[ix, b, :]
        vg[:nv, D] = 1.0
        wpack = np.concatenate(
            [W1, np.ascontiguousarray(query[q0:q0 + TQL, b, :].T), W2],
            axis=1)
        in_maps.append({
            "wpack": np.ascontiguousarray(wpack),
            "vt": np.ascontiguousarray(vg[:tve, :D].T),
            "vnp": np.ascontiguousarray(
                vg.reshape(NVC, 128, D + 1).transpose(1, 0, 2)
                .reshape(128, NVC * (D + 1)).astype(bf16_np)),
            "csp": csp,
            "ident": ident,
        })
    return in_maps, tve


def run(query, value, mask, W1, W2, scale, trace=False):
    from concourse.bass_utils import run_bass_kernel_spmd

    in_maps, tve = prep_in_maps(query, value, mask, W1, W2, scale)
    nc = get_nc(tve)
    res = run_bass_kernel_spmd(nc, in_maps, list(range(NCORES)), trace=trace)
    out = np.empty((TQ, B, D), np.float32)
    for c in range(NCORES):
        b, q0 = c // 2, (c % 2) * TQL
        out[q0:q0 + TQL, b, :] = res.results[c]["out"]
    return out, res


def kernel(query, value, mask, W1, W2, scale):
    out, _ = run(query, value, mask, W1, W2, scale, trace=False)
    return out


# revision 10
# speedup vs baseline: 1.2424x; 1.0144x over previous
"""Bahdanau (additive) attention kernel for Trainium2, 8 NeuronCores.

Full-input contract: kernel(**inputs) takes the unsharded numpy inputs and
returns the full [TQ, B, D] output. Internally shards (batch, query-half)
across 8 cores (B=4 x 2 halves of Tq), runs a Bass/Tile kernel per core via
run_bass_kernel_spmd, and reassembles.

Algorithmic core: the additive score
    scores[q,v] = sum_u s_u tanh(wq[q,u] + wk[v,u])
is evaluated via a fitted LOW-RANK SEPARABLE expansion
    tanh(a+b) ~= sum_k c_k tanh(ga_k a + da_k) * tanh(gb_k b + db_k)
Each feature contributes one accumulating PE matmul into the score PSUM.
This replaces the O(TQL*TVE*U) tanh evaluation with O(R*(TQL+TVE))
activations + O(R) matmuls; the ACT engine is the bottleneck, so the
schedule keeps it saturated: all q-side atoms run first (while the vt DMA
and wk matmul complete off the critical path), then the v-side atoms, each
immediately feeding its score matmul.

Sparsity: masked value positions are gathered on the host (mask is input
data), padded to a common TVE. Padded columns need no score penalty: their
value rows AND normalizer-column entries in vnp are zero, so whatever
exp(score) they produce contributes nothing to ctx or the normalizer.

Softmax tail: chunk-pipelined across engines -- per 128-column chunk the
score PSUM is exp'ed (ACT), PE-transposed, copied to SBUF (DVE), and
accumulated into ctx via PE matmul against [v | 1]; the last ctx column
yields the softmax normalizer consistently; DVE reciprocal + per-partition
scale on output.
"""

import sys

if "/opt/trn_rl_repo" not in sys.path:
    sys.path.insert(0, "/opt/trn_rl_repo")

import numpy as np

TQ, TV, B, D, U = 256, 1024, 4, 128, 128
NCORES = 8
TQL = 128
NEG_INF = -1e9

# Fitted separable expansion (filled from fit_structured.py):
# v-side shared atoms (gb, db)
VATOMS = [
    (0.47444510, 1.39766216),
    (0.48982596, 0.07451601),
    (1.37957418, -0.07826442),
    (1.31198537, -1.28328967),
    (1.06376755, 1.44153440),
    (0.87626874, -0.34579548),
    (1.11406505, -0.14585873),
    (1.36467373, -1.28521180),
    (-0.67345953, -2.40418053),
    (0.95521957, -2.94330263),
    (-0.61966121, 1.94561911),
    (0.16753665, 0.11532875),
    (2.26979327, 1.33731830),
    (2.12410307, 1.39618659),
    (2.58440638, 2.12091875),
    (2.37144637, 1.95568621),
    (0.38434589, -0.31339884),
    (3.02359366, -2.61771035),
    (2.54027200, -2.07246137),
]
# features: (c, ga, da, vspec); vspec = int atom idx | (i, j) atom product
FEATS = [
    (1.70959640, 0.25151360, 0.03138908, 0),
    (2.92134929, -0.43775776, 1.09167695, 1),
    (-0.77592200, -1.55171931, -1.17404127, 2),
    (0.96599334, -1.30268359, -0.93301958, 3),
    (0.53638780, 1.80897105, -1.21104717, 4),
    (1.60405707, -0.90852302, 0.42689738, 5),
    (-1.07496023, -0.62484956, 1.85881972, 6),
    (-1.96126282, -0.37054500, -0.16805789, 7),
    (-0.35367522, -1.32026529, -0.79605997, 8),
    (-0.87735492, 0.42294371, -0.19769210, 9),
    (-0.67645180, 1.32222283, -0.76137090, 10),
    (0.69646239, 1.75380993, -2.25412583, 11),
    (0.57765937, -0.71790487, -0.03128870, 12),
    (0.69881064, 1.59039760, -0.02415021, 13),
    (0.86268389, -0.23864533, -0.29861382, 14),
    (0.36266690, 1.05495822, 1.55348122, 15),
    (0.31275606, -2.06651807, -2.63506889, 16),
    (-0.25702652, 0.14892404, 0.46942550, 17),
    (-0.23037328, -1.92254341, -2.47092772, 18),
]

_CACHE = {}


def _bank_pieces(tve):
    """Split [0, tve) into PSUM-bank-aligned matmul slices (<=512 each)."""
    pieces = []
    a = 0
    while a < tve:
        n = min(512, tve - a)
        pieces.append((a, n))
        a += n
    return pieces


def _build_nc(tve):
    import concourse.bacc as bacc
    import concourse.mybir as mybir
    import concourse.tile as tile
    from contextlib import ExitStack

    f32 = mybir.dt.float32
    f32r = mybir.dt.float32r
    bf16 = mybir.dt.bfloat16
    AFT = mybir.ActivationFunctionType

    nc = bacc.Bacc("TRN2", target_bir_lowering=False, debug=False,
                   num_devices=NCORES)

    R = len(FEATS)
    NVC = -(-tve // 128)
    pieces = _bank_pieces(tve)

    wpack = nc.dram_tensor("wpack", [D, 3 * 128], bf16,
                           kind="ExternalInput").ap()
    vt = nc.dram_tensor("vt", [D, tve], bf16, kind="ExternalInput").ap()
    vnp = nc.dram_tensor("vnp", [128, NVC * (D + 1)], bf16,
                         kind="ExternalInput").ap()
    NA = len(VATOMS)
    # columns: R x (c_k*s) | R x da_k | NA x db_i
    csp = nc.dram_tensor("csp", [U, 2 * R + NA], f32,
                         kind="ExternalInput").ap()
    ident = nc.dram_tensor("ident", [128, 128], bf16,
                           kind="ExternalInput").ap()
    out = nc.dram_tensor("out", [TQL, D], f32, kind="ExternalOutput").ap()

    with tile.TileContext(nc) as tc:
        with ExitStack() as ctx:
            consts = ctx.enter_context(tc.tile_pool(name="consts", bufs=1))
            uap = ctx.enter_context(tc.tile_pool(name="ua", bufs=3))
            ps1 = ctx.enter_context(tc.tile_pool(name="ps1", bufs=1,
                                                 space="PSUM"))
            pst = ctx.enter_context(tc.tile_pool(name="pst", bufs=1,
                                                 space="PSUM"))

            wpack_sb = consts.tile([D, 3 * 128], bf16, tag="wpack")
            w1_sb = wpack_sb[:, 0:128]
            qt_sb = wpack_sb[:, 128:256]
            w2_sb = wpack_sb[:, 256:384]
            vt_sb = consts.tile([D, tve], bf16, tag="vt")
            vnp_sb = consts.tile([128, NVC * (D + 1)], bf16, tag="vnp")
            csp_sb = consts.tile([U, 2 * R + NA], f32, tag="csp")
            id_sb = consts.tile([128, 128], bf16, tag="id")
            vb_sb = consts.tile([U, R * tve], bf16, tag="vb")
            lh_sb = consts.tile([U, R * TQL], bf16, tag="lh")

            # preload the exp/tanh ACT table during the input DMAs
            warm_in = consts.tile([128, 1], f32, tag="warm_in")
            warm_out = consts.tile([128, 1], f32, tag="warm_out")
            nc.vector.memset(warm_in[:], 0.0)
            nc.scalar.activation(warm_out[:], warm_in[:], AFT.Tanh)

            # DMA routing: wq path (wpack) first on the sync HWDGE queue,
            # vt behind it (first consumed ~3us later); csp on the scalar
            # queue (overlaps the ACT table load); tail-only tensors (vnp,
            # ident) on the gpsimd SWDGE queue, off every critical path.
            nc.sync.dma_start(wpack_sb[:], wpack[:])
            nc.sync.dma_start(vt_sb[:], vt[:])
            nc.scalar.dma_start(csp_sb[:], csp[:])
            nc.gpsimd.dma_start(vnp_sb[:], vnp[:])
            nc.gpsimd.dma_start(id_sb[:], ident[:])

            # wqT[u,q] and wkT[u,v] stay in PSUM (ACT reads PSUM cheaply)
            wq_ps = ps1.tile([U, TQL], f32, tag="wq")
            nc.tensor.matmul(wq_ps[:], lhsT=w1_sb[:], rhs=qt_sb[:])
            wk_ps = ps1.tile([U, tve], f32, tag="wk")
            for a, n in pieces:
                nc.tensor.matmul(wk_ps[:, a:a + n], lhsT=w2_sb[:],
                                 rhs=vt_sb[:, a:a + n])

            # all q-side atoms first: keeps ACT busy while vt/wk complete;
            # ua stays f32 so lh sees only one bf16 rounding (accuracy)
            for k, (c_k, ga, da, vs) in enumerate(FEATS):
                ua = uap.tile([U, TQL], f32, tag="ua")
                nc.scalar.activation(ua[:], wq_ps[:], AFT.Tanh,
                                     bias=csp_sb[:, R + k:R + k + 1],
                                     scale=float(ga))
                nc.vector.tensor_scalar_mul(lh_sb[:, k * TQL:(k + 1) * TQL],
                                            ua[:], csp_sb[:, k:k + 1])

            # v-side atoms, each immediately feeding its score matmuls
            scores_ps = ps1.tile([TQL, tve], f32, tag="scores")
            for k, (c_k, ga, da, vs) in enumerate(FEATS):
                vb_k = vb_sb[:, k * tve:(k + 1) * tve]
                if isinstance(vs, tuple):
                    i, j = vs
                    nc.vector.tensor_mul(
                        vb_k, vb_sb[:, i * tve:(i + 1) * tve],
                        vb_sb[:, j * tve:(j + 1) * tve])
                else:
                    gb, db = VATOMS[vs]
                    nc.scalar.activation(
                        vb_k, wk_ps[:], AFT.Tanh,
                        bias=csp_sb[:, 2 * R + vs:2 * R + vs + 1],
                        scale=float(gb))
                lw = lh_sb[:, k * TQL:(k + 1) * TQL]
                for a, n in pieces:
                    nc.tensor.matmul(scores_ps[:, a:a + n], lhsT=lw,
                                     rhs=vb_sb[:, k * tve + a:k * tve + a + n],
                                     start=(k == 0), stop=(k == R - 1),
                                     skip_group_check=True)

            # softmax tail: piece-wise exp to bf16 (wide ACT instrs amortize
            # the ~200ns ACT overhead), then per-128-chunk bf16 transpose
            # (PE, 1 cyc/row) -> copy to SBUF (DVE 2x mode) -> single-pass
            # bf16 ctx matmul vs [v | 1] (f32 rhs would need 2 PE passes);
            # the ones column gives the softmax normalizer consistently
            exp_sb = consts.tile([TQL, NVC * 128], bf16, tag="exp")
            tp_all = pst.tile([128, NVC * 128], bf16, tag="tpa")
            et_all = consts.tile([128, NVC * 128], bf16, tag="eta")
            ctx_ps = ps1.tile([TQL, D + 1], f32, tag="ctx")
            for a, n in pieces:
                nc.scalar.activation(exp_sb[:, a:a + n],
                                     scores_ps[:, a:a + n], AFT.Exp)
            for kc in range(NVC):
                n = min(128, tve - kc * 128)
                c0 = kc * 128
                nc.tensor.transpose(tp_all[:n, c0:c0 + 128],
                                    exp_sb[:, c0:c0 + n], id_sb[:])
                nc.vector.tensor_copy(et_all[:n, c0:c0 + 128],
                                      tp_all[:n, c0:c0 + 128])
            for kc in range(NVC):
                n = min(128, tve - kc * 128)
                nc.tensor.matmul(
                    ctx_ps[:], lhsT=et_all[:n, kc * 128:kc * 128 + 128],
                    rhs=vnp_sb[:n, kc * (D + 1):(kc + 1) * (D + 1)],
                    start=(kc == 0), stop=(kc == NVC - 1))

            rins = consts.tile([TQL, 1], f32, tag="rins")
            nc.vector.reciprocal(rins[:], ctx_ps[:, D:D + 1])
            out_sb = consts.tile([TQL, D], f32, tag="out")
            nc.vector.tensor_scalar_mul(out_sb[:], ctx_ps[:, 0:D], rins[:])
            nc.sync.dma_start(out[:], out_sb[:])

    nc.compile()
    return nc


def get_nc(tve=TV):
    key = ("nc", tve)
    if key not in _CACHE:
        _CACHE[key] = _build_nc(tve)
    return _CACHE[key]


def prep_in_maps(query, value, mask, W1, W2, scale):
    """Gather valid value positions per batch; returns (in_maps, tve)."""
    query = np.asarray(query, dtype=np.float32)
    value = np.asarray(value, dtype=np.float32)
    mask = np.asarray(mask)
    W1 = np.ascontiguousarray(np.asarray(W1, dtype=np.float32))
    W2 = np.ascontiguousarray(np.asarray(W2, dtype=np.float32))
    scale = np.asarray(scale, dtype=np.float32)

    R = len(FEATS)
    NA = len(VATOMS)
    idxs = [np.nonzero(mask[:, b])[0] for b in range(B)]
    nv_max = max(1, max(len(ix) for ix in idxs))
    tve = min(TV, -(-nv_max // 4) * 4)
    NVC = -(-tve // 128)

    import ml_dtypes
    bf16_np = np.dtype(ml_dtypes.bfloat16)
    ident = np.eye(128, dtype=bf16_np)
    csp = np.zeros((U, 2 * R + NA), np.float32)
    for k, f in enumerate(FEATS):
        csp[:, k] = scale * f[0]      # (c_k * s) lhsT scale
        csp[:, R + k] = f[2]          # da_k bias
    for i, (gb, db) in enumerate(VATOMS):
        csp[:, 2 * R + i] = db        # db_i bias
    csp = np.ascontiguousarray(csp)

    in_maps = []
    for c in range(NCORES):
        b, q0 = c // 2, (c % 2) * TQL
        ix = idxs[b]
        nv = len(ix)
        # padded rows are zero in BOTH the value block and the normalizer
        # column, so they contribute nothing regardless of their score
        vg = np.zeros((NVC * 128, D + 1), np.float32)
        vg[:nv, :D] = value[ix, b, :]
        vg[:nv, D] = 1.0
        wpack = np.concatenate(
            [W1, np.ascontiguousarray(query[q0:q0 + TQL, b, :].T), W2],
            axis=1)
        in_maps.append({
            "wpack": np.ascontiguousarray(wpack.astype(bf16_np)),
            "vt": np.ascontiguousarray(vg[:tve, :D].T.astype(bf16_np)),
            "vnp": np.ascontiguousarray(
                vg.reshape(NVC, 128, D + 1).transpose(1, 0, 2)
                .reshape(128, NVC * (D + 1)).astype(bf16_np)),
            "csp": csp,
            "ident": ident,
        })
    return in_maps, tve


def run(query, value, mask, W1, W2, scale, trace=False):
    from concourse.bass_utils import run_bass_kernel_spmd

    in_maps, tve = prep_in_maps(query, value, mask, W1, W2, scale)
    nc = get_nc(tve)
    res = run_bass_kernel_spmd(nc, in_maps, list(range(NCORES)), trace=trace)
    out = np.empty((TQ, B, D), np.float32)
    for c in range(NCORES):
        b, q0 = c // 2, (c % 2) * TQL
        out[q0:q0 + TQL, b, :] = res.results[c]["out"]
    return out, res


def kernel(query, value, mask, W1, W2, scale):
    out, _ = run(query, value, mask, W1, W2, scale, trace=False)
    return out


# revision 15
# speedup vs baseline: 1.3123x; 1.0563x over previous
"""Bahdanau (additive) attention kernel for Trainium2, 8 NeuronCores.

Full-input contract: kernel(**inputs) takes the unsharded numpy inputs and
returns the full [TQ, B, D] output. Internally shards (batch, query-half)
across 8 cores (B=4 x 2 halves of Tq), runs a Bass/Tile kernel per core via
run_bass_kernel_spmd, and reassembles.

Algorithmic core: the additive score
    scores[q,v] = sum_u s_u tanh(wq[q,u] + wk[v,u])
is evaluated via a fitted LOW-RANK SEPARABLE expansion
    tanh(a+b) ~= sum_k c_k tanh(ga_k a + da_k) * tanh(gb_k b + db_k)
Each feature contributes one accumulating PE matmul into the score PSUM.
This replaces the O(TQL*TVE*U) tanh evaluation with O(R*(TQL+TVE))
activations + O(R) matmuls; the ACT engine is the bottleneck, so the
schedule keeps it saturated: all q-side atoms run first (while the vt DMA
and wk matmul complete off the critical path), then the v-side atoms, each
immediately feeding its score matmul.

Sparsity: masked value positions are gathered on the host (mask is input
data), padded to a common TVE. Padded columns need no score penalty: their
value rows AND normalizer-column entries in vnp are zero, so whatever
exp(score) they produce contributes nothing to ctx or the normalizer.

Softmax tail: chunk-pipelined across engines -- per 128-column chunk the
score PSUM is exp'ed (ACT), PE-transposed, copied to SBUF (DVE), and
accumulated into ctx via PE matmul against [v | 1]; the last ctx column
yields the softmax normalizer consistently; DVE reciprocal + per-partition
scale on output.
"""

import sys

if "/opt/trn_rl_repo" not in sys.path:
    sys.path.insert(0, "/opt/trn_rl_repo")

import numpy as np

TQ, TV, B, D, U = 256, 1024, 4, 128, 128
NCORES = 8
TQL = 128
NEG_INF = -1e9

# Fitted separable expansion (filled from fit_structured.py):
# v-side shared atoms (gb, db)
VATOMS = [
    (0.47444510, 1.39766216),
    (0.48982596, 0.07451601),
    (1.37957418, -0.07826442),
    (1.31198537, -1.28328967),
    (1.06376755, 1.44153440),
    (0.87626874, -0.34579548),
    (1.11406505, -0.14585873),
    (1.36467373, -1.28521180),
    (-0.67345953, -2.40418053),
    (0.95521957, -2.94330263),
    (-0.61966121, 1.94561911),
    (0.16753665, 0.11532875),
    (2.26979327, 1.33731830),
    (2.12410307, 1.39618659),
    (2.58440638, 2.12091875),
    (2.37144637, 1.95568621),
    (0.38434589, -0.31339884),
    (3.02359366, -2.61771035),
    (2.54027200, -2.07246137),
]
# features: (c, ga, da, vspec); vspec = int atom idx | (i, j) atom product
FEATS = [
    (1.70959640, 0.25151360, 0.03138908, 0),
    (2.92134929, -0.43775776, 1.09167695, 1),
    (-0.77592200, -1.55171931, -1.17404127, 2),
    (0.96599334, -1.30268359, -0.93301958, 3),
    (0.53638780, 1.80897105, -1.21104717, 4),
    (1.60405707, -0.90852302, 0.42689738, 5),
    (-1.07496023, -0.62484956, 1.85881972, 6),
    (-1.96126282, -0.37054500, -0.16805789, 7),
    (-0.35367522, -1.32026529, -0.79605997, 8),
    (-0.87735492, 0.42294371, -0.19769210, 9),
    (-0.67645180, 1.32222283, -0.76137090, 10),
    (0.69646239, 1.75380993, -2.25412583, 11),
    (0.57765937, -0.71790487, -0.03128870, 12),
    (0.69881064, 1.59039760, -0.02415021, 13),
    (0.86268389, -0.23864533, -0.29861382, 14),
    (0.36266690, 1.05495822, 1.55348122, 15),
    (0.31275606, -2.06651807, -2.63506889, 16),
    (-0.25702652, 0.14892404, 0.46942550, 17),
    (-0.23037328, -1.92254341, -2.47092772, 18),
]

_CACHE = {}


def _bank_pieces(tve):
    """Split [0, tve) into PSUM-bank-aligned matmul slices (<=512 each)."""
    pieces = []
    a = 0
    while a < tve:
        n = min(512, tve - a)
        pieces.append((a, n))
        a += n
    return pieces


def _build_nc(tve):
    import concourse.bacc as bacc
    import concourse.mybir as mybir
    import concourse.tile as tile
    from contextlib import ExitStack

    f32 = mybir.dt.float32
    f32r = mybir.dt.float32r
    bf16 = mybir.dt.bfloat16
    AFT = mybir.ActivationFunctionType

    nc = bacc.Bacc("TRN2", target_bir_lowering=False, debug=False,
                   num_devices=NCORES)

    R = len(FEATS)
    NVC = -(-tve // 128)
    pieces = _bank_pieces(tve)

    wpack = nc.dram_tensor("wpack", [D, 3 * 128], bf16,
                           kind="ExternalInput").ap()
    vt = nc.dram_tensor("vt", [D, tve], bf16, kind="ExternalInput").ap()
    vnp = nc.dram_tensor("vnp", [128, NVC * (D + 1)], bf16,
                         kind="ExternalInput").ap()
    NA = len(VATOMS)
    # columns: R x (c_k*s) | R x da_k | NA x db_i
    csp = nc.dram_tensor("csp", [U, 2 * R + NA], f32,
                         kind="ExternalInput").ap()
    ident = nc.dram_tensor("ident", [128, 128], bf16,
                           kind="ExternalInput").ap()
    # unnormalized [ctx | normalizer]; the division happens on the host
    out = nc.dram_tensor("out", [TQL, D + 1], f32,
                         kind="ExternalOutput").ap()

    with tile.TileContext(nc) as tc:
        with ExitStack() as ctx:
            consts = ctx.enter_context(tc.tile_pool(name="consts", bufs=1))
            uap = ctx.enter_context(tc.tile_pool(name="ua", bufs=3))
            ps1 = ctx.enter_context(tc.tile_pool(name="ps1", bufs=1,
                                                 space="PSUM"))
            # ping-pong transpose tiles: per-chunk tiles break the false
            # WAR serialization between transpose kc+1 and copy kc
            pst = ctx.enter_context(tc.tile_pool(name="pst", bufs=2,
                                                 space="PSUM"))

            wpack_sb = consts.tile([D, 3 * 128], bf16, tag="wpack")
            w1_sb = wpack_sb[:, 0:128]
            qt_sb = wpack_sb[:, 128:256]
            w2_sb = wpack_sb[:, 256:384]
            vt_sb = consts.tile([D, tve], bf16, tag="vt")
            vnp_sb = consts.tile([128, NVC * (D + 1)], bf16, tag="vnp")
            csp_sb = consts.tile([U, 2 * R + NA], f32, tag="csp")
            id_sb = consts.tile([128, 128], bf16, tag="id")
            vb_sb = consts.tile([U, R * tve], bf16, tag="vb")
            lh_sb = consts.tile([U, R * TQL], bf16, tag="lh")

            # preload the exp/tanh ACT table during the input DMAs
            warm_in = consts.tile([128, 1], f32, tag="warm_in")
            warm_out = consts.tile([128, 1], f32, tag="warm_out")
            nc.vector.memset(warm_in[:], 0.0)
            nc.scalar.activation(warm_out[:], warm_in[:], AFT.Tanh)

            # DMA routing: wq path (wpack) first on the sync HWDGE queue,
            # vt behind it (first consumed ~3us later); csp on the scalar
            # queue (overlaps the ACT table load); tail-only tensors (vnp,
            # ident) on the gpsimd SWDGE queue, off every critical path.
            nc.sync.dma_start(wpack_sb[:], wpack[:])
            nc.sync.dma_start(vt_sb[:], vt[:])
            nc.scalar.dma_start(csp_sb[:], csp[:])
            nc.gpsimd.dma_start(vnp_sb[:], vnp[:])
            nc.gpsimd.dma_start(id_sb[:], ident[:])

            # wqT[u,q] and wkT[u,v] stay in PSUM (ACT reads PSUM cheaply)
            wq_ps = ps1.tile([U, TQL], f32, tag="wq")
            nc.tensor.matmul(wq_ps[:], lhsT=w1_sb[:], rhs=qt_sb[:])
            wk_ps = ps1.tile([U, tve], f32, tag="wk")
            for a, n in pieces:
                nc.tensor.matmul(wk_ps[:, a:a + n], lhsT=w2_sb[:],
                                 rhs=vt_sb[:, a:a + n])

            # all q-side atoms first: keeps ACT busy while vt/wk complete;
            # ua stays f32 so lh sees only one bf16 rounding (accuracy)
            for k, (c_k, ga, da, vs) in enumerate(FEATS):
                ua = uap.tile([U, TQL], f32, tag="ua")
                nc.scalar.activation(ua[:], wq_ps[:], AFT.Tanh,
                                     bias=csp_sb[:, R + k:R + k + 1],
                                     scale=float(ga))
                nc.vector.tensor_scalar_mul(lh_sb[:, k * TQL:(k + 1) * TQL],
                                            ua[:], csp_sb[:, k:k + 1])

            # v-side atoms, each immediately feeding its score matmuls
            scores_ps = ps1.tile([TQL, tve], f32, tag="scores")
            for k, (c_k, ga, da, vs) in enumerate(FEATS):
                vb_k = vb_sb[:, k * tve:(k + 1) * tve]
                if isinstance(vs, tuple):
                    i, j = vs
                    nc.vector.tensor_mul(
                        vb_k, vb_sb[:, i * tve:(i + 1) * tve],
                        vb_sb[:, j * tve:(j + 1) * tve])
                else:
                    gb, db = VATOMS[vs]
                    nc.scalar.activation(
                        vb_k, wk_ps[:], AFT.Tanh,
                        bias=csp_sb[:, 2 * R + vs:2 * R + vs + 1],
                        scale=float(gb))
                lw = lh_sb[:, k * TQL:(k + 1) * TQL]
                for a, n in pieces:
                    nc.tensor.matmul(scores_ps[:, a:a + n], lhsT=lw,
                                     rhs=vb_sb[:, k * tve + a:k * tve + a + n],
                                     start=(k == 0), stop=(k == R - 1),
                                     skip_group_check=True)

            # softmax tail: piece-wise exp to bf16 (wide ACT instrs amortize
            # the ~200ns ACT overhead), then per-128-chunk bf16 transpose
            # (PE, 1 cyc/row) -> copy to SBUF (DVE 2x mode) -> single-pass
            # bf16 ctx matmul vs [v | 1] (f32 rhs would need 2 PE passes);
            # the ones column gives the softmax normalizer consistently
            exp_sb = consts.tile([TQL, NVC * 128], bf16, tag="exp")
            et_all = consts.tile([128, NVC * 128], bf16, tag="eta")
            ctx_ps = ps1.tile([TQL, D + 1], f32, tag="ctx")
            for a, n in pieces:
                nc.scalar.activation(exp_sb[:, a:a + n],
                                     scores_ps[:, a:a + n], AFT.Exp)
            for kc in range(NVC):
                n = min(128, tve - kc * 128)
                c0 = kc * 128
                tp = pst.tile([128, 128], bf16, tag="tp")
                nc.tensor.transpose(tp[:n, :], exp_sb[:, c0:c0 + n], id_sb[:])
                nc.vector.tensor_copy(et_all[:n, c0:c0 + 128], tp[:n, :])
            for kc in range(NVC):
                n = min(128, tve - kc * 128)
                nc.tensor.matmul(
                    ctx_ps[:], lhsT=et_all[:n, kc * 128:kc * 128 + 128],
                    rhs=vnp_sb[:n, kc * (D + 1):(kc + 1) * (D + 1)],
                    start=(kc == 0), stop=(kc == NVC - 1))
            out_sb = consts.tile([TQL, D + 1], f32, tag="out")
            nc.vector.tensor_copy(out_sb[:], ctx_ps[:])
            nc.sync.dma_start(out[:], out_sb[:])

    nc.compile()
    return nc


def get_nc(tve=TV):
    key = ("nc", tve)
    if key not in _CACHE:
        _CACHE[key] = _build_nc(tve)
    return _CACHE[key]


def prep_in_maps(query, value, mask, W1, W2, scale):
    """Gather valid value positions per batch; returns (in_maps, tve)."""
    query = np.asarray(query, dtype=np.float32)
    value = np.asarray(value, dtype=np.float32)
    mask = np.asarray(mask)
    W1 = np.ascontiguousarray(np.asarray(W1, dtype=np.float32))
    W2 = np.ascontiguousarray(np.asarray(W2, dtype=np.float32))
    scale = np.asarray(scale, dtype=np.float32)

    R = len(FEATS)
    NA = len(VATOMS)
    idxs = [np.nonzero(mask[:, b])[0] for b in range(B)]
    nv_max = max(1, max(len(ix) for ix in idxs))
    tve = min(TV, -(-nv_max // 4) * 4)
    NVC = -(-tve // 128)

    import ml_dtypes
    bf16_np = np.dtype(ml_dtypes.bfloat16)
    ident = np.eye(128, dtype=bf16_np)
    csp = np.zeros((U, 2 * R + NA), np.float32)
    for k, f in enumerate(FEATS):
        csp[:, k] = scale * f[0]      # (c_k * s) lhsT scale
        csp[:, R + k] = f[2]          # da_k bias
    for i, (gb, db) in enumerate(VATOMS):
        csp[:, 2 * R + i] = db        # db_i bias
    csp = np.ascontiguousarray(csp)

    in_maps = []
    for c in range(NCORES):
        b, q0 = c // 2, (c % 2) * TQL
        ix = idxs[b]
        nv = len(ix)
        # padded rows are zero in BOTH the value block and the normalizer
        # column, so they contribute nothing regardless of their score
        vg = np.zeros((NVC * 128, D + 1), np.float32)
        vg[:nv, :D] = value[ix, b, :]
        vg[:nv, D] = 1.0
        wpack = np.concatenate(
            [W1, np.ascontiguousarray(query[q0:q0 + TQL, b, :].T), W2],
            axis=1)
        in_maps.append({
            "wpack": np.ascontiguousarray(wpack.astype(bf16_np)),
            "vt": np.ascontiguousarray(vg[:tve, :D].T.astype(bf16_np)),
            "vnp": np.ascontiguousarray(
                vg.reshape(NVC, 128, D + 1).transpose(1, 0, 2)
                .reshape(128, NVC * (D + 1)).astype(bf16_np)),
            "csp": csp,
            "ident": ident,
        })
    return in_maps, tve


def run(query, value, mask, W1, W2, scale, trace=False):
    from concourse.bass_utils import run_bass_kernel_spmd

    in_maps, tve = prep_in_maps(query, value, mask, W1, W2, scale)
    nc = get_nc(tve)
    res = run_bass_kernel_spmd(nc, in_maps, list(range(NCORES)), trace=trace)
    out = np.empty((TQ, B, D), np.float32)
    for c in range(NCORES):
        b, q0 = c // 2, (c % 2) * TQL
        cn = np.asarray(res.results[c]["out"], np.float64)
        out[q0:q0 + TQL, b, :] = cn[:, :D] / cn[:, D:D + 1]
    return out, res


def kernel(query, value, mask, W1, W2, scale):
    out, _ = run(query, value, mask, W1, W2, scale, trace=False)
    return out


# revision 18
# speedup vs baseline: 1.3980x; 1.0653x over previous
"""Bahdanau (additive) attention kernel for Trainium2, 8 NeuronCores.

Full-input contract: kernel(**inputs) takes the unsharded numpy inputs and
returns the full [TQ, B, D] output. Internally shards (batch, value-half)
across 8 cores: each core handles ALL TQ=256 queries of one batch and HALF
of that batch's mask-gathered value positions (sequence parallel). Cores
return unnormalized [ctx | normalizer] partial sums; the host adds the two
halves and divides (exact same math as a single softmax).

Algorithmic core: the additive score
    scores[q,v] = sum_u s_u tanh(wq[q,u] + wk[v,u])
is evaluated via a fitted LOW-RANK SEPARABLE expansion
    tanh(a+b) ~= sum_k c_k tanh(ga_k a + da_k) * tanh(gb_k b + db_k)
Each feature contributes accumulating PE matmuls into the score PSUM.
This replaces the O(TQ*TVE*U) tanh evaluation with O(R*(TQ+TVE))
activations + O(R) matmuls; the ACT engine is the bottleneck, and the
v-split sharding minimizes per-core ACT columns (256+~280 vs 128+~550).

Sparsity: masked value positions are gathered on the host (mask is input
data), split between the core pair, padded to a common TVE. Padded columns
need no score penalty: their value rows AND normalizer-column entries in
vnp are zero, so whatever exp(score) they produce contributes nothing.

Softmax tail: per-q-half piece-wise exp to bf16 (wide ACT instrs), then
per-128-chunk bf16 PE transpose into ping-pong PSUM tiles, DVE copy to
SBUF, single-pass bf16 ctx matmuls vs [v | 1]; [ctx | norm] DMAs out
unnormalized (host divides in f64).
"""

import sys

if "/opt/trn_rl_repo" not in sys.path:
    sys.path.insert(0, "/opt/trn_rl_repo")

import numpy as np

TQ, TV, B, D, U = 256, 1024, 4, 128, 128
NCORES = 8
NEG_INF = -1e9

# Fitted separable expansion:
# v-side shared atoms (gb, db)
VATOMS = [
    (0.47444510, 1.39766216),
    (0.48982596, 0.07451601),
    (1.37957418, -0.07826442),
    (1.31198537, -1.28328967),
    (1.06376755, 1.44153440),
    (0.87626874, -0.34579548),
    (1.11406505, -0.14585873),
    (1.36467373, -1.28521180),
    (-0.67345953, -2.40418053),
    (0.95521957, -2.94330263),
    (-0.61966121, 1.94561911),
    (0.16753665, 0.11532875),
    (2.26979327, 1.33731830),
    (2.12410307, 1.39618659),
    (2.58440638, 2.12091875),
    (2.37144637, 1.95568621),
    (0.38434589, -0.31339884),
    (3.02359366, -2.61771035),
    (2.54027200, -2.07246137),
]
# features: (c, ga, da, vspec); vspec = int atom idx | (i, j) atom product
FEATS = [
    (1.70959640, 0.25151360, 0.03138908, 0),
    (2.92134929, -0.43775776, 1.09167695, 1),
    (-0.77592200, -1.55171931, -1.17404127, 2),
    (0.96599334, -1.30268359, -0.93301958, 3),
    (0.53638780, 1.80897105, -1.21104717, 4),
    (1.60405707, -0.90852302, 0.42689738, 5),
    (-1.07496023, -0.62484956, 1.85881972, 6),
    (-1.96126282, -0.37054500, -0.16805789, 7),
    (-0.35367522, -1.32026529, -0.79605997, 8),
    (-0.87735492, 0.42294371, -0.19769210, 9),
    (-0.67645180, 1.32222283, -0.76137090, 10),
    (0.69646239, 1.75380993, -2.25412583, 11),
    (0.57765937, -0.71790487, -0.03128870, 12),
    (0.69881064, 1.59039760, -0.02415021, 13),
    (0.86268389, -0.23864533, -0.29861382, 14),
    (0.36266690, 1.05495822, 1.55348122, 15),
    (0.31275606, -2.06651807, -2.63506889, 16),
    (-0.25702652, 0.14892404, 0.46942550, 17),
    (-0.23037328, -1.92254341, -2.47092772, 18),
]

_CACHE = {}


def _build_nc(tve):
    import concourse.bacc as bacc
    import concourse.mybir as mybir
    import concourse.tile as tile
    from contextlib import ExitStack

    f32 = mybir.dt.float32
    bf16 = mybir.dt.bfloat16
    AFT = mybir.ActivationFunctionType

    nc = bacc.Bacc("TRN2", target_bir_lowering=False, debug=False,
                   num_devices=NCORES)

    R = len(FEATS)
    NA = len(VATOMS)
    NVC = -(-tve // 128)

    # [w1 | qt(256) | w2]
    wpack = nc.dram_tensor("wpack", [D, 128 + TQ + 128], bf16,
                           kind="ExternalInput").ap()
    vt = nc.dram_tensor("vt", [D, tve], bf16, kind="ExternalInput").ap()
    vnp = nc.dram_tensor("vnp", [128, NVC * (D + 1)], bf16,
                         kind="ExternalInput").ap()
    # columns: R x (c_k*s) | R x da_k | NA x db_i
    csp = nc.dram_tensor("csp", [U, 2 * R + NA], f32,
                         kind="ExternalInput").ap()
    ident = nc.dram_tensor("ident", [128, 128], bf16,
                           kind="ExternalInput").ap()
    # unnormalized [ctx | normalizer] per q-half; host adds halves + divides
    out = nc.dram_tensor("out", [TQ, D + 1], f32,
                         kind="ExternalOutput").ap()

    with tile.TileContext(nc) as tc:
        with ExitStack() as ctx:
            consts = ctx.enter_context(tc.tile_pool(name="consts", bufs=1))
            ps1 = ctx.enter_context(tc.tile_pool(name="ps1", bufs=1,
                                                 space="PSUM"))
            # ping-pong transpose tiles: per-chunk tiles break the false
            # WAR serialization between transpose kc+1 and copy kc
            pst = ctx.enter_context(tc.tile_pool(name="pst", bufs=2,
                                                 space="PSUM"))

            wpack_sb = consts.tile([D, 128 + TQ + 128], bf16, tag="wpack")
            w1_sb = wpack_sb[:, 0:128]
            qt_sb = wpack_sb[:, 128:128 + TQ]
            w2_sb = wpack_sb[:, 128 + TQ:]
            vt_sb = consts.tile([D, tve], bf16, tag="vt")
            vnp_sb = consts.tile([128, NVC * (D + 1)], bf16, tag="vnp")
            csp_sb = consts.tile([U, 2 * R + NA], f32, tag="csp")
            id_sb = consts.tile([128, 128], bf16, tag="id")
            vb_sb = consts.tile([U, R * tve], bf16, tag="vb")
            lh_sb = consts.tile([U, R * TQ], bf16, tag="lh")
            ua_sb = consts.tile([U, R * TQ], f32, tag="ua")

            # preload the exp/tanh ACT table during the input DMAs
            warm_in = consts.tile([128, 1], f32, tag="warm_in")
            warm_out = consts.tile([128, 1], f32, tag="warm_out")
            nc.vector.memset(warm_in[:], 0.0)
            nc.scalar.activation(warm_out[:], warm_in[:], AFT.Tanh)

            # DMA routing: the wq path ([w1|qt], 96KB bf16) leads the sync
            # HWDGE queue so the first matmul fires ~1us earlier than with
            # one monolithic wpack DMA; w2+vt follow (first consumed several
            # us later); csp on the scalar queue (overlaps the ACT table
            # load); tail-only tensors (vnp, ident) on the gpsimd SWDGE
            # queue, off every critical path.
            nc.sync.dma_start(wpack_sb[:, 0:128 + TQ], wpack[:, 0:128 + TQ])
            nc.sync.dma_start(wpack_sb[:, 128 + TQ:], wpack[:, 128 + TQ:])
            nc.sync.dma_start(vt_sb[:], vt[:])
            nc.scalar.dma_start(csp_sb[:], csp[:])
            nc.gpsimd.dma_start(vnp_sb[:], vnp[:])
            nc.gpsimd.dma_start(id_sb[:], ident[:])

            # wqT[u,q] and wkT[u,v] stay in PSUM (ACT reads PSUM cheaply)
            wq_ps = ps1.tile([U, TQ], f32, tag="wq")
            nc.tensor.matmul(wq_ps[:], lhsT=w1_sb[:], rhs=qt_sb[:])
            wk_ps = ps1.tile([U, tve], f32, tag="wk")
            nc.tensor.matmul(wk_ps[:], lhsT=w2_sb[:], rhs=vt_sb[:])

            # q-side atoms (emitted first; the scheduler interleaves them
            # with v-atoms as their score matmuls allow). Distinct ua tiles
            # give it full lookahead while vt/wk complete off-path.
            for k, (c_k, ga, da, vs) in enumerate(FEATS):
                ua = ua_sb[:, k * TQ:(k + 1) * TQ]
                nc.scalar.activation(ua, wq_ps[:], AFT.Tanh,
                                     bias=csp_sb[:, R + k:R + k + 1],
                                     scale=float(ga))
                nc.vector.tensor_scalar_mul(lh_sb[:, k * TQ:(k + 1) * TQ],
                                            ua, csp_sb[:, k:k + 1])

            # v-side atoms, each immediately feeding its two score matmuls
            # (one per q-half PSUM tile)
            sc0_ps = ps1.tile([128, tve], f32, tag="scores0")
            sc1_ps = ps1.tile([128, tve], f32, tag="scores1")
            sc_ps = [sc0_ps, sc1_ps]
            for k, (c_k, ga, da, vs) in enumerate(FEATS):
                vb_k = vb_sb[:, k * tve:(k + 1) * tve]
                if isinstance(vs, tuple):
                    i, j = vs
                    nc.vector.tensor_mul(
                        vb_k, vb_sb[:, i * tve:(i + 1) * tve],
                        vb_sb[:, j * tve:(j + 1) * tve])
                else:
                    gb, db = VATOMS[vs]
                    nc.scalar.activation(
                        vb_k, wk_ps[:], AFT.Tanh,
                        bias=csp_sb[:, 2 * R + vs:2 * R + vs + 1],
                        scale=float(gb))
                for h in range(2):
                    lw = lh_sb[:, k * TQ + h * 128:k * TQ + (h + 1) * 128]
                    nc.tensor.matmul(sc_ps[h][:], lhsT=lw, rhs=vb_k,
                                     start=(k == 0), stop=(k == R - 1),
                                     skip_group_check=True)

            # softmax tail per q-half: piece-wise exp to bf16, per-chunk
            # bf16 transpose (ping-pong PSUM) -> DVE copy -> single-pass
            # bf16 ctx matmul vs [v | 1]; ones column = softmax normalizer
            exp_sb = consts.tile([128, 2 * NVC * 128], bf16, tag="exp")
            et_all = consts.tile([128, 2 * NVC * 128], bf16, tag="eta")
            ctx0_ps = ps1.tile([128, D + 1], f32, tag="ctx0")
            ctx1_ps = ps1.tile([128, D + 1], f32, tag="ctx1")
            ctx_ps = [ctx0_ps, ctx1_ps]
            for h in range(2):
                nc.scalar.activation(exp_sb[:, h * NVC * 128:
                                            h * NVC * 128 + tve],
                                     sc_ps[h][:], AFT.Exp)
            for h in range(2):
                for kc in range(NVC):
                    n = min(128, tve - kc * 128)
                    c0 = h * NVC * 128 + kc * 128
                    tp = pst.tile([128, 128], bf16, tag="tp")
                    nc.tensor.transpose(
                        tp[:n, :], exp_sb[:, c0:c0 + n], id_sb[:])
                    nc.vector.tensor_copy(et_all[:n, c0:c0 + 128],
                                          tp[:n, :])
            for h in range(2):
                for kc in range(NVC):
                    n = min(128, tve - kc * 128)
                    c0 = h * NVC * 128 + kc * 128
                    nc.tensor.matmul(
                        ctx_ps[h][:], lhsT=et_all[:n, c0:c0 + 128],
                        rhs=vnp_sb[:n, kc * (D + 1):(kc + 1) * (D + 1)],
                        start=(kc == 0), stop=(kc == NVC - 1))
            out_sb = consts.tile([128, 2 * (D + 1)], f32, tag="out")
            for h in range(2):
                nc.vector.tensor_copy(
                    out_sb[:, h * (D + 1):(h + 1) * (D + 1)], ctx_ps[h][:])
                nc.sync.dma_start(out[h * 128:(h + 1) * 128, :],
                                  out_sb[:, h * (D + 1):(h + 1) * (D + 1)])

    nc.compile()
    return nc


def get_nc(tve):
    key = ("nc", tve)
    if key not in _CACHE:
        _CACHE[key] = _build_nc(tve)
    return _CACHE[key]


def prep_in_maps(query, value, mask, W1, W2, scale):
    """Gather valid value positions per batch, split per core pair."""
    import ml_dtypes

    query = np.asarray(query, dtype=np.float32)
    value = np.asarray(value, dtype=np.float32)
    mask = np.asarray(mask)
    W1 = np.ascontiguousarray(np.asarray(W1, dtype=np.float32))
    W2 = np.ascontiguousarray(np.asarray(W2, dtype=np.float32))
    scale = np.asarray(scale, dtype=np.float32)

    R = len(FEATS)
    NA = len(VATOMS)
    bf16_np = np.dtype(ml_dtypes.bfloat16)

    idxs = []
    for b in range(B):
        ix = np.nonzero(mask[:, b])[0]
        h = (len(ix) + 1) // 2
        idxs.append((ix[:h], ix[h:]))
    nv_max = max(1, max(len(ixh) for pair in idxs for ixh in pair))
    tve = -(-nv_max // 4) * 4
    NVC = -(-tve // 128)

    ident = np.eye(128, dtype=bf16_np)
    csp = np.zeros((U, 2 * R + NA), np.float32)
    for k, f in enumerate(FEATS):
        csp[:, k] = scale * f[0]      # (c_k * s) lhsT scale
        csp[:, R + k] = f[2]          # da_k bias
    for i, (gb, db) in enumerate(VATOMS):
        csp[:, 2 * R + i] = db        # db_i bias
    csp = np.ascontiguousarray(csp)

    in_maps = []
    for c in range(NCORES):
        b, half = c // 2, c % 2
        ix = idxs[b][half]
        nv = len(ix)
        # padded rows are zero in BOTH the value block and the normalizer
        # column, so they contribute nothing regardless of their score
        vg = np.zeros((NVC * 128, D + 1), np.float32)
        vg[:nv, :D] = value[ix, b, :]
        vg[:nv, D] = 1.0
        wpack = np.concatenate(
            [W1, np.ascontiguousarray(query[:, b, :].T), W2], axis=1)
        in_maps.append({
            "wpack": np.ascontiguousarray(wpack.astype(bf16_np)),
            "vt": np.ascontiguousarray(vg[:tve, :D].T.astype(bf16_np)),
            "vnp": np.ascontiguousarray(
                vg.reshape(NVC, 128, D + 1).transpose(1, 0, 2)
                .reshape(128, NVC * (D + 1)).astype(bf16_np)),
            "csp": csp,
            "ident": ident,
        })
    return in_maps, tve


def run(query, value, mask, W1, W2, scale, trace=False):
    from concourse.bass_utils import run_bass_kernel_spmd

    in_maps, tve = prep_in_maps(query, value, mask, W1, W2, scale)
    nc = get_nc(tve)
    res = run_bass_kernel_spmd(nc, in_maps, list(range(NCORES)), trace=trace)
    out = np.empty((TQ, B, D), np.float32)
    for b in range(B):
        ca = np.asarray(res.results[2 * b]["out"], np.float64)
        cb = np.asarray(res.results[2 * b + 1]["out"], np.float64)
        s = ca + cb
        out[:, b, :] = s[:, :D] / s[:, D:D + 1]
    return out, res


def kernel(query, value, mask, W1, W2, scale):
    out, _ = run(query, value, mask, W1, W2, scale, trace=False)
    return out
